# revision 15
# baseline (speedup 1.0000x reference)
"""Mixtral-style MoE (T=16384, H=1024, F=3584, E=8, top-2) on 8 TRN2 NeuronCores.

Sharding strategy: expert parallel. Core e owns expert e's weights. The host
computes the (tiny) router gate + top-2 + renormalized softmax, shards tokens
by expert id (the dispatch), and each core runs its expert's SwiGLU FFN over
its gathered tokens in bf16 (fp32 PSUM accumulation), applying the per-token
routing weight on PSUM eviction. The host scatter-adds the two expert
contributions per token back into the full [T, H] output (the combine).
"""

import os
import numpy as np

T, H, F, E, TOP_K = 16384, 1024, 3584, 8, 2
P = 128
N_CORES = 8

_last_exec_ns = None
_last_trace_path = None


# ---------------------------------------------------------------------------
# Device kernel body (per core): y = diag(scale) @ (silu(x@w1.T) * (x@w3.T)) @ w2.T
# ---------------------------------------------------------------------------

def _ffn_body(tc, xg, w1t, w3t, w2t, scale, y, C, Hd, Fd):
    """w1t/w3t/w2t are pre-sharded by the host into the blocked-transposed
    bf16 layout the tensor engine consumes directly:
      w1t/w3t: [n_f, P, Hd] with w1t[ft, p, k*P+fi] = w1[ft*P+fi, k*P+p]
      w2t:     [n_f, P, Hd] with w2t[ft, f, h] = w2[h, ft*P+f]
    """
    from contextlib import ExitStack

    import concourse.mybir as mybir
    from concourse.masks import make_identity

    nc = tc.nc
    BF = mybir.dt.bfloat16
    F32 = mybir.dt.float32
    SIGMOID = mybir.ActivationFunctionType.Sigmoid

    n_k = Hd // P          # contraction tiles over H for GEMM1/3
    n_f = Fd // P          # tiles over F
    assert C % P == 0 and Hd % P == 0 and Fd % P == 0
    n_cs_total = C // P

    # token tiles: chunks of up to 512 (4 x 128-row subtiles)
    ctiles = []
    c0 = 0
    while c0 < C:
        cw = min(512, C - c0)
        ctiles.append((c0, cw))
        c0 += cw
    # output column chunks of up to 512 for GEMM2
    htiles = []
    h0 = 0
    while h0 < Hd:
        hw = min(512, Hd - h0)
        htiles.append((h0, hw))
        h0 += hw

    with ExitStack() as ctx:
        const = ctx.enter_context(tc.tile_pool(name="const", bufs=1))
        ident = const.tile([P, P], BF)
        make_identity(nc, ident)

        # routing-weight scale, resident: partition = token % 128, col = token // 128
        scale_sb = const.tile([P, n_cs_total], F32)
        nc.sync.dma_start(
            scale_sb[:], scale.rearrange("(s p) o -> p (s o)", p=P)
        )

        psum_t = ctx.enter_context(tc.tile_pool(name="psum_t", bufs=2, space="PSUM"))

        # ---- main loop over token tiles -----------------------------------
        w2t_pool = ctx.enter_context(tc.tile_pool(name="w2t", bufs=n_f))
        xt_pool = ctx.enter_context(tc.tile_pool(name="xt", bufs=2 * n_k))
        g_pool = ctx.enter_context(tc.tile_pool(name="g", bufs=n_f + 2))
        w_pool = ctx.enter_context(tc.tile_pool(name="wstream", bufs=6))
        s_pool = ctx.enter_context(tc.tile_pool(name="spool", bufs=3))
        y_pool = ctx.enter_context(tc.tile_pool(name="ypool", bufs=3))
        stage = ctx.enter_context(tc.tile_pool(name="stageC", bufs=3))
        psum_mm = ctx.enter_context(tc.tile_pool(name="psum_mm", bufs=2, space="PSUM"))
        psum_y = ctx.enter_context(tc.tile_pool(name="psum_y", bufs=2, space="PSUM"))

        def build_xT(c0, cw):
            ncs = cw // P
            xT = [xt_pool.tile([P, 512], BF, tag="xt", name=f"xT_{k}") for k in range(n_k)]
            for cs in range(ncs):
                rowC = stage.tile([P, Hd], F32, tag="c_f32", name="rowC")
                nc.sync.dma_start(rowC[:], xg[c0 + cs * P: c0 + (cs + 1) * P, :])
                rowC_bf = stage.tile([P, Hd], BF, tag="c_bf", name="rowC_bf")
                nc.vector.tensor_copy(rowC_bf[:], rowC[:])
                for k in range(n_k):
                    ptC = psum_t.tile([P, P], BF, tag="pt", name="ptC")
                    nc.tensor.transpose(ptC[:], rowC_bf[:, k * P:(k + 1) * P], ident[:])
                    nc.vector.tensor_copy(xT[k][:, cs * P:(cs + 1) * P], ptC[:])
            return xT

        xT = build_xT(*ctiles[0])

        # w2T resident in SBUF (bf16) — issued after the first token tile's
        # input DMAs so they don't delay the first matmuls
        w2T = []
        for i in range(n_f):
            t = w2t_pool.tile([P, Hd], BF, tag="w2t", name=f"w2T_{i}")
            nc.sync.dma_start(t[:], w2t[i])
            w2T.append(t)

        for ci, (c0, cw) in enumerate(ctiles):
            ncs = cw // P
            # GEMM1 + GEMM3 -> G tiles [P(f), cw] bf16
            G = []
            for ft in range(n_f):
                w1ts = w_pool.tile([P, Hd], BF, tag="w1t", name="w1ts")
                nc.sync.dma_start(w1ts[:], w1t[ft])
                w3ts = w_pool.tile([P, Hd], BF, tag="w3t", name="w3ts")
                nc.sync.dma_start(w3ts[:], w3t[ft])
                ps1 = psum_mm.tile([P, 512], F32, tag="ps1", name="ps1")
                ps3 = psum_mm.tile([P, 512], F32, tag="ps3", name="ps3")
                for k in range(n_k):
                    nc.tensor.matmul(
                        ps1[:, :cw], w1ts[:, k * P:(k + 1) * P], xT[k][:, :cw],
                        start=(k == 0), stop=(k == n_k - 1),
                    )
                for k in range(n_k):
                    nc.tensor.matmul(
                        ps3[:, :cw], w3ts[:, k * P:(k + 1) * P], xT[k][:, :cw],
                        start=(k == 0), stop=(k == n_k - 1),
                    )
                s1 = s_pool.tile([P, 512], BF, tag="s1", name="s1")
                nc.scalar.activation(s1[:, :cw], ps1[:, :cw], SIGMOID)
                t1 = s_pool.tile([P, 512], BF, tag="t1", name="t1")
                nc.vector.tensor_mul(t1[:, :cw], s1[:, :cw], ps1[:, :cw])
                g = g_pool.tile([P, 512], BF, tag="g", name="g")
                nc.vector.tensor_mul(g[:, :cw], t1[:, :cw], ps3[:, :cw])
                G.append(g)

            # prefetch + pre-transpose next token tile while GEMM2 runs
            next_xT = build_xT(*ctiles[ci + 1]) if ci + 1 < len(ctiles) else None

            # GEMM2: y[c0+ms*P : .., :] = scale * (G.T @ w2T)
            for ms in range(ncs):
                col = c0 // P + ms
                for h0, hw in htiles:
                    psy = psum_y.tile([P, 512], F32, tag="psy", name="psy")
                    for ft in range(n_f):
                        nc.tensor.matmul(
                            psy[:, :hw], G[ft][:, ms * P:(ms + 1) * P],
                            w2T[ft][:, h0:h0 + hw],
                            start=(ft == 0), stop=(ft == n_f - 1),
                        )
                    yrow = y_pool.tile([P, 512], F32, tag="y", name="yrow")
                    nc.vector.tensor_scalar_mul(
                        yrow[:, :hw], psy[:, :hw], scale_sb[:, col:col + 1]
                    )
                    nc.sync.dma_start(
                        y[c0 + ms * P: c0 + (ms + 1) * P, h0:h0 + hw], yrow[:, :hw]
                    )
            xT = next_xT


# ---------------------------------------------------------------------------
# Program build + SPMD run
# ---------------------------------------------------------------------------

def _build_program(C, Hd=H, Fd=F):
    import concourse.mybir as mybir
    import concourse.tile as tile
    from concourse import bacc

    nc = bacc.Bacc("TRN2", target_bir_lowering=False, debug=False, num_devices=1)
    f32 = mybir.dt.float32
    bf16 = mybir.dt.bfloat16
    n_f = Fd // P
    xg = nc.dram_tensor("xg", [C, Hd], f32, kind="ExternalInput").ap()
    w1t = nc.dram_tensor("w1t", [n_f, P, Hd], bf16, kind="ExternalInput").ap()
    w3t = nc.dram_tensor("w3t", [n_f, P, Hd], bf16, kind="ExternalInput").ap()
    w2t = nc.dram_tensor("w2t", [n_f, P, Hd], bf16, kind="ExternalInput").ap()
    sc = nc.dram_tensor("scale", [C, 1], f32, kind="ExternalInput").ap()
    y = nc.dram_tensor("y", [C, Hd], f32, kind="ExternalOutput").ap()
    with tile.TileContext(nc) as tc:
        _ffn_body(tc, xg, w1t, w3t, w2t, sc, y, C, Hd, Fd)
    nc.compile()
    return nc


def _pack_w13(w, Fd, Hd):
    """[F, H] f32 -> [n_f, P, Hd] bf16 with out[ft, p, k*P+fi] = w[ft*P+fi, k*P+p]."""
    import ml_dtypes
    n_f, n_k = Fd // P, Hd // P
    a = np.ascontiguousarray(w).reshape(n_f, P, n_k, P)      # [ft, fi, k, p]
    a = a.transpose(0, 3, 2, 1).reshape(n_f, P, Hd)          # [ft, p, (k fi)]
    return np.ascontiguousarray(a.astype(ml_dtypes.bfloat16))


def _pack_w2(w, Fd, Hd):
    """[H, F] f32 -> [n_f, P, Hd] bf16 with out[ft, f, h] = w[h, ft*P+f]."""
    import ml_dtypes
    n_f = Fd // P
    a = np.ascontiguousarray(w).reshape(Hd, n_f, P)          # [h, ft, f]
    a = a.transpose(1, 2, 0)                                 # [ft, f, h]
    return np.ascontiguousarray(a.astype(ml_dtypes.bfloat16))


def _maybe_enable_trace():
    """Register the axon NTFF profiling hook (missing antenv.axon_hooks shim)."""
    import sys
    import types

    try:
        import antenv
        if "antenv.axon_hooks" not in sys.modules:
            mod = types.ModuleType("antenv.axon_hooks")
            hook = [None]
            mod.set_axon_ntff_profile_hook = lambda h: hook.__setitem__(0, h)
            mod.get_axon_ntff_profile_hook = lambda: hook[0]
            sys.modules["antenv.axon_hooks"] = mod
            antenv.axon_hooks = mod
            from trn_agent_boot.trn_boot import _ntff_profile_via_ctypes
            mod.set_axon_ntff_profile_hook(
                _ntff_profile_via_ctypes("/opt/axon/libaxon_pjrt.so")
            )
        from concourse import bass_utils
        bass_utils.upload_artifacts = lambda tmpdir: "/tmp/no_upload"
        return True
    except Exception as e:  # profiling is best-effort
        print(f"[kernel] trace setup failed: {e}")
        return False


# ---------------------------------------------------------------------------
# Host routing + entry point
# ---------------------------------------------------------------------------

def kernel(hidden_states, gate_w, w1, w3, w2):
    global _last_exec_ns, _last_trace_path
    from concourse.bass_utils import run_bass_kernel_spmd

    x = np.ascontiguousarray(np.asarray(hidden_states, dtype=np.float32))
    gw = np.asarray(gate_w, dtype=np.float32)
    w1 = np.ascontiguousarray(np.asarray(w1, dtype=np.float32))
    w3 = np.ascontiguousarray(np.asarray(w3, dtype=np.float32))
    w2 = np.ascontiguousarray(np.asarray(w2, dtype=np.float32))
    Tn = x.shape[0]

    # Router gate on host (float64 for selection stability), top-2 + renorm softmax
    logits = x.astype(np.float64) @ gw.astype(np.float64).T           # [T, E]
    top2 = np.argsort(-logits, axis=1, kind="stable")[:, :TOP_K]      # [T, 2] desc
    v = np.take_along_axis(logits, top2, axis=1)                      # [T, 2]
    vmax = v[:, :1]
    ev = np.exp(v - vmax)
    wts = (ev / ev.sum(axis=1, keepdims=True)).astype(np.float32)     # [T, 2]

    # Shard tokens by expert id
    tok_lists, wt_lists = [], []
    for e in range(E):
        sel = np.nonzero((top2 == e).any(axis=1))[0]
        we = np.where(top2[sel, 0] == e, wts[sel, 0], wts[sel, 1])
        tok_lists.append(sel)
        wt_lists.append(we.astype(np.float32))
    max_count = max(len(s) for s in tok_lists)
    C = ((max_count + P - 1) // P) * P

    in_maps = []
    for e in range(E):
        sel, we = tok_lists[e], wt_lists[e]
        n = len(sel)
        xg = np.zeros((C, H), np.float32)
        xg[:n] = x[sel]
        scale = np.zeros((C, 1), np.float32)
        scale[:n, 0] = we
        in_maps.append({
            "xg": xg,
            "w1t": _pack_w13(w1[e], F, H),
            "w3t": _pack_w13(w3[e], F, H),
            "w2t": _pack_w2(w2[e], F, H),
            "scale": scale,
        })

    trace = os.environ.get("BASS_MOE_TRACE", "0") == "1"
    if trace:
        trace = _maybe_enable_trace()

    nc = _build_program(C)
    res = run_bass_kernel_spmd(
        nc, in_maps, core_ids=list(range(N_CORES)), trace=trace
    )
    _last_exec_ns = res.exec_time_ns
    if res.instructions_and_trace is not None:
        _last_trace_path = res.instructions_and_trace[1]
    if trace and _last_exec_ns is not None:
        print(f"HW exec time: {_last_exec_ns} ns")
        print(f"trace: {_last_trace_path}")

    # Combine: scatter-add the two expert contributions per token
    out = np.zeros((Tn, H), np.float32)
    for e in range(E):
        sel = tok_lists[e]
        out[sel] += res.results[e]["y"][:len(sel)]
    return out
